# revision 1
# baseline (speedup 1.0000x reference)
"""AdaAtt attention kernel for Trainium2 (8 NeuronCores, data-parallel over batch).

Self-contained: takes full inputs (as produced by the problem's setup_inputs),
shards batch across 8 cores, runs a Bass/Tile kernel via run_bass_kernel_spmd,
and returns the full [256, 2048] float32 output.
"""

from contextlib import ExitStack

import ml_dtypes
import numpy as np

import concourse.bass as bass
import concourse.mybir as mybir
import concourse.tile as tile
from concourse import bacc
from concourse.bass_utils import run_bass_kernel_spmd
from concourse.masks import make_identity

# Problem dims (hardcoded per spec)
B, A, D = 256, 196, 2048
NCORES = 8
BC = B // NCORES          # 32 batch rows per core
P = 128
KC = D // P               # 16 feature chunks
NS = D // 512             # 4 psum n-slices
A1 = A - P                # 68 rows in second conv chunk
SLOTS = A + 1             # 197 attention slots (slot 0 = fake region)
GP_CHUNKS = 0             # bias-add chunks handled by gpsimd (rest on DVE);
                          # gpsimd tensor_scalar measured ~3us/op and stalls DVE
                          # via the shared SBUF port -- keep everything on DVE

F32 = mybir.dt.float32
BF16 = mybir.dt.bfloat16
FP8 = mybir.dt.float8e4
AFT = mybir.ActivationFunctionType
ALU = mybir.AluOpType

_CACHE = {}


def _build_graph():
    nc = bacc.Bacc("TRN2")

    # ---------------- DRAM parameters ----------------
    xfr_d = nc.dram_tensor("xfr", [P, KC * BC], BF16, kind="ExternalInput")
    xho_d = nc.dram_tensor("xho", [P, KC * BC], BF16, kind="ExternalInput")
    cfe_d = nc.dram_tensor("cfe", [BC, P, KC * A], FP8, kind="ExternalInput")
    cf_d = nc.dram_tensor("cf", [BC, A, D], BF16, kind="ExternalInput")
    w_d = {
        name: nc.dram_tensor(name, [P, KC * D], BF16, kind="ExternalInput")
        for name in ["wfr", "wfre", "who", "whoe", "wa2h"]
    }
    b_d = {
        name: nc.dram_tensor(name, [1, D], BF16, kind="ExternalInput")
        for name in ["bfr", "bfre", "bho", "bhoe", "ba2h"]
    }
    wal_d = nc.dram_tensor("walpha", [P, KC], BF16, kind="ExternalInput")
    out_d = nc.dram_tensor("out", [BC, D], F32, kind="ExternalOutput")

    with ExitStack() as ctx:
        tc = ctx.enter_context(tile.TileContext(nc))

        singles = ctx.enter_context(tc.tile_pool(name="singles", bufs=1))
        wpool = ctx.enter_context(tc.tile_pool(name="wpool", bufs=8))
        bmpool = ctx.enter_context(tc.tile_pool(name="bm", bufs=2))
        cfepool = ctx.enter_context(tc.tile_pool(name="cfep", bufs=4))
        habpool = ctx.enter_context(tc.tile_pool(name="habp", bufs=3))
        tanpool = ctx.enter_context(tc.tile_pool(name="tanp", bufs=3))
        cfpool = ctx.enter_context(tc.tile_pool(name="cfp", bufs=4))
        misc = ctx.enter_context(tc.tile_pool(name="misc", bufs=2))
        rowpool = ctx.enter_context(tc.tile_pool(name="rows", bufs=4))

        mpsum = ctx.enter_context(tc.tile_pool(name="mpsum", bufs=4, space="PSUM"))
        tpsum = ctx.enter_context(tc.tile_pool(name="tpsum", bufs=2, space="PSUM"))
        spsum = ctx.enter_context(tc.tile_pool(name="spsum", bufs=2, space="PSUM"))

        # ---------------- constants / small inputs ----------------
        ones = singles.tile([1, P], BF16, tag="ones")
        nc.vector.memset(ones[:], 1.0)
        id_bf = singles.tile([BC, BC], BF16, tag="id_bf")
        make_identity(nc, id_bf[:])
        id_f32 = singles.tile([BC, BC], F32, tag="id_f32")
        make_identity(nc, id_f32[:])

        xfr = singles.tile([P, KC * BC], BF16, tag="xfr")
        nc.sync.dma_start(xfr[:], xfr_d[:])
        xho = singles.tile([P, KC * BC], BF16, tag="xho")
        nc.sync.dma_start(xho[:], xho_d[:])
        wal = singles.tile([P, KC], BF16, tag="wal")
        nc.sync.dma_start(wal[:], wal_d[:])
        b_sb = {}
        for name in b_d:
            t = singles.tile([1, D], BF16, tag=name)
            nc.sync.dma_start(t[:], b_d[name][:])
            b_sb[name] = t

        # ---------------- helpers ----------------
        def linear_batch_m(x_lhsT, wname, bname, act, out_dtype=BF16):
            """out_bm[BC, D] = act(x @ W.T + b); x given as feature-major chunks
            x_lhsT [P, KC*BC] (chunk k = x.T[k*128:(k+1)*128, :]).  Weights are
            streamed per k-chunk ([128, 2048] tiles) with all 4 psum n-slices
            accumulating concurrently."""
            ps = [mpsum.tile([BC, 512], F32, tag="mp", name=f"mp_{wname}{ns}") for ns in range(NS)]
            for k in range(KC):
                wt = wpool.tile([P, D], BF16, tag="w", name=f"w_{wname}{k}")
                nc.sync.dma_start(wt[:], w_d[wname][:, k * D : (k + 1) * D])
                for ns in range(NS):
                    nc.tensor.matmul(
                        ps[ns][:],
                        lhsT=x_lhsT[:, k * BC : (k + 1) * BC],
                        rhs=wt[:, ns * 512 : (ns + 1) * 512],
                        start=(k == 0),
                        stop=False,
                    )
            bm = bmpool.tile([BC, D], out_dtype, tag="bm")
            for ns in range(NS):
                nc.tensor.matmul(
                    ps[ns][:],
                    lhsT=ones[0:1, 0:BC],
                    rhs=b_sb[bname][0:1, ns * 512 : (ns + 1) * 512],
                    start=False,
                    stop=True,
                )
                nc.scalar.activation(bm[:, ns * 512 : (ns + 1) * 512], ps[ns][:], act)
            return bm

        def to_feature_major(bm, name, in_f32=False, out_dtype=BF16):
            """bm [BC, D] -> fT [P, KC*BC] via PE transposes."""
            ident = id_f32 if in_f32 else id_bf
            dt = F32 if in_f32 else BF16
            fT = singles.tile([P, KC * BC], out_dtype, tag=name)
            for k in range(KC):
                pt = tpsum.tile([P, BC], dt, tag="tps", name=f"pt_{name}{k}")
                nc.tensor.transpose(pt[:], bm[:, k * P : (k + 1) * P], ident[:])
                nc.vector.tensor_copy(fT[:, k * BC : (k + 1) * BC], pt[:])
            return fT

        # ---------------- phase 1: front linears ----------------
        # ho-chain first: hoeT is the only input the fused attention loop
        # needs, so the loop overlaps the fr-chain's weight streaming.
        hol_bm = linear_batch_m(xho, "who", "bho", AFT.Tanh)
        holT = to_feature_major(hol_bm, "holT")
        hoe_bm = linear_batch_m(holT, "whoe", "bhoe", AFT.Copy)
        hoeT = to_feature_major(hoe_bm, "hoeT")
        fr_bm = linear_batch_m(xfr, "wfr", "bfr", AFT.Relu)
        frT = to_feature_major(fr_bm, "frT")
        fre_bm = linear_batch_m(frT, "wfre", "bfre", AFT.Copy)
        freT = to_feature_major(fre_bm, "freT")

        # slot-0 scores for all b: w_alpha . tanh(fre + hoe)
        ha0 = misc.tile([P, KC * BC], BF16, tag="ha0")
        nc.vector.tensor_tensor(ha0[:], freT[:], hoeT[:], op=ALU.add)
        ta0 = misc.tile([P, KC * BC], BF16, tag="ta0")
        nc.scalar.activation(ta0[:], ha0[:], AFT.Tanh)
        s0ps = spsum.tile([1, A], F32, tag="sps")
        for c in range(KC):
            nc.tensor.matmul(
                s0ps[0:1, 0:BC],
                lhsT=wal[:, c : c + 1],
                rhs=ta0[:, c * BC : (c + 1) * BC],
                start=(c == 0),
                stop=(c == KC - 1),
            )
        s0row = singles.tile([1, BC], F32, tag="s0row")
        nc.vector.tensor_copy(s0row[:], s0ps[0:1, 0:BC])
        # slot-0 exp weights for all b at once.  No max-subtraction anywhere:
        # |score| <= ||w_alpha||_1 ~ 23, so exp stays comfortably inside f32.
        e0all = singles.tile([1, BC], F32, tag="e0all")
        nc.scalar.activation(e0all[:], s0row[:], AFT.Exp)

        # ---------------- phase 2 (fused): scores -> row softmax -> vis ----------------
        # pim0/pim1 hold, per batch row b, a [128|68, 32] block whose only
        # nonzero column b is the (unnormalized) exp attention weights for the
        # conv slots; accumulating all b into shared [32, 512] psum tiles
        # yields the unnormalized vis for every batch row.
        pim0 = singles.tile([P, BC * BC], BF16, tag="pim0")
        nc.vector.memset(pim0[:], 0.0)
        pim1 = singles.tile([A1, BC * BC], BF16, tag="pim1")
        nc.vector.memset(pim1[:], 0.0)
        Zrow = singles.tile([1, BC], F32, tag="Zrow")

        vp = [mpsum.tile([BC, 512], F32, tag="mp", name=f"vp{ns}") for ns in range(NS)]

        def emit_vis(b, c0, c1):
            # vis accumulation (unnormalized exp weights, masked columns)
            for ns in range(NS):
                nc.tensor.matmul(
                    vp[ns][:],
                    lhsT=pim0[:, b * BC : (b + 1) * BC],
                    rhs=c0[:, ns * 512 : (ns + 1) * 512],
                    start=(b == 0),
                    stop=False,
                )
                nc.tensor.matmul(
                    vp[ns][:],
                    lhsT=pim1[:, b * BC : (b + 1) * BC],
                    rhs=c1[:, ns * 512 : (ns + 1) * 512],
                    start=False,
                    stop=(b == BC - 1),
                )

        # Software-pipelined by two iterations: the vis matmuls for b-2 are
        # emitted between b's score matmuls and b's softmax-dependent
        # transposes, so the tensor engine never waits on the softmax tail
        # and the pim-column copies have a full iteration of slack.
        pending_vis = []
        wa2h_pre = []
        prev_tail = []
        for b in range(BC):
            cfeb = cfepool.tile([P, KC * A], FP8, tag="cfeb")
            nc.sync.dma_start(cfeb[:], cfe_d[b])
            c0 = cfpool.tile([P, D], BF16, tag="cfb0")
            nc.sync.dma_start(c0[:], cf_d[b, 0:P, :])
            c1 = cfpool.tile([A1, D], BF16, tag="cfb1")
            nc.sync.dma_start(c1[:], cf_d[b, P:A, :])

            # bias-add as broadcast tensor_tensor (hoe column per chunk,
            # broadcast along the 196 slots) + ACT tanh, in two halves so the
            # tanh of half 0 overlaps the add of half 1
            NH = 2
            HK = KC // NH
            hab = habpool.tile([P, KC * A], BF16, tag="hab")
            th = tanpool.tile([P, KC * A], BF16, tag="th")
            for h in range(NH):
                hview = (
                    hoeT[:]
                    .rearrange("p (c q) -> p c q", q=BC)[:, h * HK : (h + 1) * HK, b : b + 1]
                    .to_broadcast((P, HK, A))
                )
                sl = slice(h * HK * A, (h + 1) * HK * A)
                nc.vector.tensor_tensor(
                    hab[:, sl].rearrange("p (c a) -> p c a", a=A),
                    cfeb[:, sl].rearrange("p (c a) -> p c a", a=A),
                    hview,
                    op=ALU.add,
                )
                nc.scalar.activation(th[:, sl], hab[:, sl], AFT.Tanh)
            sps = spsum.tile([1, A], F32, tag="sps")
            for c in range(KC):
                nc.tensor.matmul(
                    sps[:],
                    lhsT=wal[:, c : c + 1],
                    rhs=th[:, c * A : (c + 1) * A],
                    start=(c == 0),
                    stop=(c == KC - 1),
                )
            if len(pending_vis) >= 1:
                emit_vis(*pending_vis.pop(0))
            # unnormalized exp weights straight from the score psum (no max
            # subtraction -- scores are bounded); zcell = sum over conv slots
            erow = rowpool.tile([1, A], BF16, tag="erow")
            zcell = rowpool.tile([1, 1], F32, tag="zcell")
            nc.scalar.activation(erow[:], sps[:], AFT.Exp, accum_out=zcell[:])
            zi = nc.vector.tensor_copy(Zrow[0:1, b : b + 1], zcell[:])
            # transpose exp weights into the masked lhsT columns
            tp = tpsum.tile([P, 4], BF16, tag="tps", name=f"tp{b}")
            nc.tensor.transpose(tp[:, 0:1], erow[0:1, 0:P], id_bf[0:1, 0:1])
            nc.tensor.transpose(tp[0:A1, 2:3], erow[0:1, P:A], id_bf[0:1, 0:1])
            p0i = nc.vector.tensor_copy(pim0[:, b * BC + b : b * BC + b + 1], tp[:, 0:1])
            p1i = nc.vector.tensor_copy(pim1[:, b * BC + b : b * BC + b + 1], tp[0:A1, 2:3])
            prev_tail = [zi, p0i, p1i]
            pending_vis.append((b, c0, c1))
            # prefetch the first final-layer weight chunks late in the loop
            # (at most wpool bufs, so DMA lanes never head-of-line block)
            if b >= BC - 8:
                k = b - (BC - 8)
                wt = wpool.tile([P, D], BF16, tag="w", name=f"w_a2h{k}")
                nc.sync.dma_start(wt[:], w_d["wa2h"][:, k * D : (k + 1) * D])
                wa2h_pre.append(wt)
        for pv in pending_vis:
            emit_vis(*pv)

        # ---------------- phase 3: normalize, atten_out, final linear ----------------
        # total Z = conv-slot sum + slot-0 exp; 1/Z as a column for vis rows
        nc.vector.tensor_tensor(Zrow[:], Zrow[:], e0all[:], op=ALU.add)
        zt = tpsum.tile([P, 2], F32, tag="tps", name="zt")
        nc.tensor.transpose(zt[0:BC, 0:1], Zrow[:], id_f32[0:1, 0:1])
        zcol = singles.tile([BC, 1], F32, tag="zcol")
        nc.vector.tensor_copy(zcol[:], zt[0:BC, 0:1])
        rinv = singles.tile([BC, 1], F32, tag="rinv")
        nc.vector.reciprocal(rinv[:], zcol[:])
        vis_bm = singles.tile([BC, D], F32, tag="vis_bm")
        for ns in range(NS):
            nc.vector.tensor_scalar_mul(
                vis_bm[:, ns * 512 : (ns + 1) * 512], vp[ns][:], rinv[:]
            )
        # pi0 = e0/Z broadcast to [128, BC]
        zrinv = singles.tile([1, BC], F32, tag="zrinv")
        nc.vector.reciprocal(zrinv[:], Zrow[:])
        e0z = singles.tile([1, BC], BF16, tag="e0z")
        nc.vector.tensor_tensor(e0z[:], e0all[:], zrinv[:], op=ALU.mult)
        pb = tpsum.tile([P, BC], F32, tag="tps", name="pb")
        nc.tensor.matmul(pb[:], lhsT=ones[0:1, 0:P], rhs=e0z[:], start=True, stop=True)
        pi0b = singles.tile([P, BC], F32, tag="pi0b")
        nc.vector.tensor_copy(pi0b[:], pb[:])

        # atten_out.T = vis.T + hol.T + pi0*fr.T  (feature-major, bf16)
        attT = singles.tile([P, KC * BC], BF16, tag="attT")
        for k in range(KC):
            vt = tpsum.tile([P, BC], F32, tag="tps", name=f"vt{k}")
            nc.tensor.transpose(vt[:], vis_bm[:, k * P : (k + 1) * P], id_f32[:])
            t1 = misc.tile([P, BC], F32, tag="t1")
            nc.vector.tensor_tensor(t1[:], vt[:], holT[:, k * BC : (k + 1) * BC], op=ALU.add)
            t2 = misc.tile([P, BC], F32, tag="t2")
            nc.vector.tensor_tensor(t2[:], pi0b[:], frT[:, k * BC : (k + 1) * BC], op=ALU.mult)
            nc.vector.tensor_tensor(
                attT[:, k * BC : (k + 1) * BC], t1[:], t2[:], op=ALU.add
            )

        # final linear: out = tanh(atten @ W_a2h.T + b)
        ps = [mpsum.tile([BC, 512], F32, tag="mp", name=f"fps{ns}") for ns in range(NS)]
        for k in range(KC):
            if k < len(wa2h_pre):
                wt = wa2h_pre[k]
            else:
                wt = wpool.tile([P, D], BF16, tag="w", name=f"w_a2h{k}")
                nc.sync.dma_start(wt[:], w_d["wa2h"][:, k * D : (k + 1) * D])
            for ns in range(NS):
                nc.tensor.matmul(
                    ps[ns][:],
                    lhsT=attT[:, k * BC : (k + 1) * BC],
                    rhs=wt[:, ns * 512 : (ns + 1) * 512],
                    start=(k == 0),
                    stop=False,
                )
        outsb = singles.tile([BC, D], F32, tag="outsb")
        for ns in range(NS):
            nc.tensor.matmul(
                ps[ns][:],
                lhsT=ones[0:1, 0:BC],
                rhs=b_sb["ba2h"][0:1, ns * 512 : (ns + 1) * 512],
                start=False,
                stop=True,
            )
            nc.scalar.activation(outsb[:, ns * 512 : (ns + 1) * 512], ps[ns][:], AFT.Tanh)
        nc.sync.dma_start(out_d[:], outsb[:])

    nc.compile()
    return nc


def _bf16(x):
    return np.ascontiguousarray(np.asarray(x, dtype=np.float32).astype(ml_dtypes.bfloat16))


def _prep_weight(W):
    # W [D, D] (out_features, in_features) -> W.T chunk layout [128, KC*D]
    Wt = np.asarray(W, dtype=np.float32).T  # [k, n]
    return _bf16(Wt.reshape(KC, P, D).transpose(1, 0, 2).reshape(P, KC * D))


def _prep_xT(x):
    # x [BC, D] -> X.T chunk layout [128, KC*BC]
    t = np.asarray(x, dtype=np.float32).T  # [D, BC]
    return _bf16(t.reshape(KC, P, BC).transpose(1, 0, 2).reshape(P, KC * BC))


def _prep_cfe(e):
    # e [BC, A, D] -> [BC, 128, KC*A] with chunk c at free offset c*A, fp8
    t = np.asarray(e, dtype=np.float32).transpose(0, 2, 1)  # [BC, D, A]
    r = t.reshape(BC, KC, P, A).transpose(0, 2, 1, 3).reshape(BC, P, KC * A)
    return np.ascontiguousarray(r.astype(ml_dtypes.float8_e4m3))


def _make_in_maps(inputs):
    h_out = np.asarray(inputs["h_out"], dtype=np.float32)
    fake_region = np.asarray(inputs["fake_region"], dtype=np.float32)
    conv_feat = np.asarray(inputs["conv_feat"], dtype=np.float32)
    conv_feat_embed = np.asarray(inputs["conv_feat_embed"], dtype=np.float32)

    shared = {
        "wfr": _prep_weight(inputs["W_fr"]),
        "wfre": _prep_weight(inputs["W_fre"]),
        "who": _prep_weight(inputs["W_ho"]),
        "whoe": _prep_weight(inputs["W_hoe"]),
        "wa2h": _prep_weight(inputs["W_a2h"]),
        "bfr": _bf16(np.asarray(inputs["b_fr"]).reshape(1, D)),
        "bfre": _bf16(np.asarray(inputs["b_fre"]).reshape(1, D)),
        "bho": _bf16(np.asarray(inputs["b_ho"]).reshape(1, D)),
        "bhoe": _bf16(np.asarray(inputs["b_hoe"]).reshape(1, D)),
        "ba2h": _bf16(np.asarray(inputs["b_a2h"]).reshape(1, D)),
        "walpha": _bf16(np.asarray(inputs["w_alpha"]).reshape(KC, P).T),
    }
    in_maps = []
    for c in range(NCORES):
        sl = slice(c * BC, (c + 1) * BC)
        in_maps.append(
            dict(
                shared,
                xfr=_prep_xT(fake_region[sl]),
                xho=_prep_xT(h_out[sl]),
                cfe=_prep_cfe(conv_feat_embed[sl]),
                cf=_bf16(conv_feat[sl]),
            )
        )
    return in_maps


def _run(inputs, trace=False):
    if "nc" not in _CACHE:
        _CACHE["nc"] = _build_graph()
    nc = _CACHE["nc"]
    in_maps = _make_in_maps(inputs)
    res = run_bass_kernel_spmd(nc, in_maps, core_ids=list(range(NCORES)), trace=trace)
    out = np.concatenate([r["out"] for r in res.results], axis=0)
    return out.astype(np.float32), res


def kernel(**inputs):
    out, _ = _run(inputs, trace=False)
    return out



# revision 14
# speedup vs baseline: 1.1265x; 1.1265x over previous
"""AdaAtt attention kernel for Trainium2 (8 NeuronCores, data-parallel over batch).

v2: fp8-heavy. Per-core HBM bytes cut from ~81MB to ~56MB:
  - conv_feat fp8 (x16 scale), conv_feat_embed fp8 (x16)
  - W_fr / W_fre / W_hoe fp8 (x64) driven in DoubleRow perf mode
  - W_ho / W_a2h stay bf16 (fp8 there fails the 2e-2 tolerance)
  - tanh'd score operand + exp attention weights fp8 -> DoubleRow scores/vis
  - bias-add of hoe into conv_feat_embed fused into the ACT tanh (bias arg),
    eliminating the 12.8M-element DVE broadcast add

Self-contained: takes full inputs (as produced by the problem's setup_inputs),
shards batch across 8 cores, runs a Bass/Tile kernel via run_bass_kernel_spmd,
and returns the full [256, 2048] float32 output.
"""

from contextlib import ExitStack

import ml_dtypes
import numpy as np

import concourse.bass as bass
import concourse.mybir as mybir
import concourse.tile as tile
from concourse import bacc
from concourse.bass_utils import run_bass_kernel_spmd
from concourse.masks import make_identity

# Problem dims (hardcoded per spec)
B, A, D = 256, 196, 2048
NCORES = 8
BC = B // NCORES          # 32 batch rows per core
P = 128
KC = D // P               # 16 feature chunks
NP = KC // 2              # 8 DoubleRow chunk pairs
NS = D // 512             # 4 psum n-slices
A1 = A - P                # 68 rows in second conv chunk

XS = 16.0                 # fp8 scale for inputs/activations
WS = 64.0                 # fp8 scale for weights

F32 = mybir.dt.float32
BF16 = mybir.dt.bfloat16
FP8 = mybir.dt.float8e4
AFT = mybir.ActivationFunctionType
ALU = mybir.AluOpType
DR = mybir.MatmulPerfMode.DoubleRow

_CACHE = {}


def _build_graph():
    nc = bacc.Bacc("TRN2")

    # ---------------- DRAM parameters ----------------
    xfr_d = nc.dram_tensor("xfr", [P, KC * BC], FP8, kind="ExternalInput")   # 16*fake_region.T
    xho_d = nc.dram_tensor("xho", [P, KC * BC], BF16, kind="ExternalInput")  # h_out.T
    cfe_d = nc.dram_tensor("cfe", [BC, P, KC * A], FP8, kind="ExternalInput")  # 16*cfe
    cf_d = nc.dram_tensor("cf", [BC, A, D], FP8, kind="ExternalInput")         # 16*cf
    w8_d = {
        name: nc.dram_tensor(name, [P, KC * D], FP8, kind="ExternalInput")     # 64*W.T
        for name in ["wfr", "wfre", "whoe"]
    }
    wb_d = {
        name: nc.dram_tensor(name, [P, KC * D], BF16, kind="ExternalInput")    # W.T
        for name in ["who", "wa2h"]
    }
    # biases pre-scaled on host to match their psum scales
    b_d = {
        name: nc.dram_tensor(name, [1, D], BF16, kind="ExternalInput")
        for name in ["bfr", "bfre", "bho", "bhoe", "ba2h"]
    }
    # w_alpha padded to 16 cols per chunk (col 0 real, rest zero): DoubleRow
    # ldweights requires k-tile step % 16 == 0
    wal_d = nc.dram_tensor("walpha", [P, KC * 16], FP8, kind="ExternalInput")  # 64*w_alpha
    out_d = nc.dram_tensor("out", [BC, D], F32, kind="ExternalOutput")

    with ExitStack() as ctx:
        tc = ctx.enter_context(tile.TileContext(nc))

        singles = ctx.enter_context(tc.tile_pool(name="singles", bufs=1))
        wpool = ctx.enter_context(tc.tile_pool(name="wpool", bufs=8))
        bmpool = ctx.enter_context(tc.tile_pool(name="bm", bufs=2))
        cfepool = ctx.enter_context(tc.tile_pool(name="cfep", bufs=3))
        thpool = ctx.enter_context(tc.tile_pool(name="thp", bufs=3))
        cfpool = ctx.enter_context(tc.tile_pool(name="cfp", bufs=3))
        misc = ctx.enter_context(tc.tile_pool(name="misc", bufs=2))
        rowpool = ctx.enter_context(tc.tile_pool(name="rows", bufs=4))

        mpsum = ctx.enter_context(tc.tile_pool(name="mpsum", bufs=4, space="PSUM"))
        tpsum = ctx.enter_context(tc.tile_pool(name="tpsum", bufs=2, space="PSUM"))
        spsum = ctx.enter_context(tc.tile_pool(name="spsum", bufs=2, space="PSUM"))

        # ---------------- constants / small inputs ----------------
        ones = singles.tile([1, P], BF16, tag="ones")
        nc.vector.memset(ones[:], 1.0)
        id_bf = singles.tile([BC, BC], BF16, tag="id_bf")
        make_identity(nc, id_bf[:])
        id_f32 = singles.tile([BC, BC], F32, tag="id_f32")
        make_identity(nc, id_f32[:])

        xfr = singles.tile([P, KC * BC], FP8, tag="xfr")
        nc.sync.dma_start(xfr[:], xfr_d[:])
        xho = singles.tile([P, KC * BC], BF16, tag="xho")
        nc.sync.dma_start(xho[:], xho_d[:])
        wal = singles.tile([P, KC * 16], FP8, tag="wal")
        nc.sync.dma_start(wal[:], wal_d[:])
        b_sb = {}
        for name in b_d:
            t = singles.tile([1, D], BF16, tag=name)
            nc.sync.dma_start(t[:], b_d[name][:])
            b_sb[name] = t

        # ---------------- helpers ----------------
        def linear_bf16(x_lhsT, wname, bname, act, out_bm, scale=1.0):
            """out_bm[BC, D] = act((x @ W.T + b)*scale); bf16 weights streamed
            per k-chunk, all 4 psum n-slices accumulating concurrently."""
            ps = [mpsum.tile([BC, 512], F32, tag="mp", name=f"mp_{wname}{ns}") for ns in range(NS)]
            for k in range(KC):
                wt = wpool.tile([P, D], BF16, tag="w", name=f"w_{wname}{k}")
                nc.sync.dma_start(wt[:], wb_d[wname][:, k * D : (k + 1) * D])
                for ns in range(NS):
                    nc.tensor.matmul(
                        ps[ns][:],
                        lhsT=x_lhsT[:, k * BC : (k + 1) * BC],
                        rhs=wt[:, ns * 512 : (ns + 1) * 512],
                        start=(k == 0),
                        stop=False,
                    )
            for ns in range(NS):
                nc.tensor.matmul(
                    ps[ns][:],
                    lhsT=ones[0:1, 0:BC],
                    rhs=b_sb[bname][0:1, ns * 512 : (ns + 1) * 512],
                    start=False,
                    stop=True,
                )
                nc.scalar.activation(out_bm[:, ns * 512 : (ns + 1) * 512], ps[ns][:], act, scale=scale)
            return out_bm

        def linear_dr(x8_lhsT, wname, bname, act, out_bm, scale):
            """out_bm[BC, D] = act((psum + b)*scale) with fp8 DoubleRow matmuls.
            x8_lhsT [P, KC*BC] fp8, weight [P, KC*D] fp8; psum = xscale*wscale*(x@W.T)."""
            ps = [mpsum.tile([BC, 512], F32, tag="mp", name=f"mp_{wname}{ns}") for ns in range(NS)]
            for p in range(NP):
                wt = wpool.tile([P, 2 * D], FP8, tag="w", name=f"w_{wname}{p}")
                nc.sync.dma_start(wt[:], w8_d[wname][:, 2 * p * D : (2 * p + 2) * D])
                wv = wt[:].rearrange("q (k n) -> q k n", n=D)
                xv = x8_lhsT[:, 2 * p * BC : (2 * p + 2) * BC].rearrange(
                    "q (k m) -> q k m", m=BC
                )
                for ns in range(NS):
                    nc.tensor.matmul(
                        ps[ns][:],
                        lhsT=xv,
                        rhs=wv[:, :, ns * 512 : (ns + 1) * 512],
                        start=(p == 0),
                        stop=False,
                        perf_mode=DR,
                    )
            for ns in range(NS):
                nc.tensor.matmul(
                    ps[ns][:],
                    lhsT=ones[0:1, 0:BC],
                    rhs=b_sb[bname][0:1, ns * 512 : (ns + 1) * 512],
                    start=False,
                    stop=True,
                )
                nc.scalar.activation(out_bm[:, ns * 512 : (ns + 1) * 512], ps[ns][:], act, scale=scale)
            return out_bm

        def to_feature_major(bm, outs, in_f32=False):
            """bm [BC, D] -> feature-major [P, KC*BC] via PE transposes.
            outs: list of (tile, mul) to produce (mul applied on DVE, dtype
            conversion via the copy)."""
            ident = id_f32 if in_f32 else id_bf
            dt = F32 if in_f32 else BF16
            for k in range(KC):
                pt = tpsum.tile([P, BC], dt, tag="tps", name=f"pt_{outs[0][0].name}{k}")
                nc.tensor.transpose(pt[:], bm[:, k * P : (k + 1) * P], ident[:])
                for t, mul in outs:
                    if mul == 1.0:
                        nc.vector.tensor_copy(t[:, k * BC : (k + 1) * BC], pt[:])
                    else:
                        nc.vector.tensor_scalar_mul(
                            t[:, k * BC : (k + 1) * BC], pt[:], mul
                        )

        # ---------------- phase 1: front linears ----------------
        # ho-chain first: hoeT is the only input the fused attention loop needs.
        hol_bm = bmpool.tile([BC, D], BF16, tag="bm", name="hol_bm")
        linear_bf16(xho, "who", "bho", AFT.Tanh, hol_bm)
        holT = singles.tile([P, KC * BC], BF16, tag="holT")
        hol8T = singles.tile([P, KC * BC], FP8, tag="hol8T")   # 16*hol
        to_feature_major(hol_bm, [(holT, 1.0), (hol8T, XS)])

        hoe_bm = bmpool.tile([BC, D], BF16, tag="bm", name="hoe_bm")
        linear_dr(hol8T, "whoe", "bhoe", AFT.Copy, hoe_bm, scale=1.0 / (XS * WS))
        hoeT = singles.tile([P, KC * BC], BF16, tag="hoeT")
        to_feature_major(hoe_bm, [(hoeT, 1.0)])

        fr_bm = bmpool.tile([BC, D], BF16, tag="bm", name="fr_bm")   # 16*fr
        linear_dr(xfr, "wfr", "bfr", AFT.Relu, fr_bm, scale=1.0 / WS)
        frT8 = singles.tile([P, KC * BC], FP8, tag="frT8")           # 16*fr
        to_feature_major(fr_bm, [(frT8, 1.0)])

        fre_bm = bmpool.tile([BC, D], BF16, tag="bm", name="fre_bm")
        linear_dr(frT8, "wfre", "bfre", AFT.Copy, fre_bm, scale=1.0 / (XS * WS))
        freT = singles.tile([P, KC * BC], BF16, tag="freT")
        to_feature_major(fre_bm, [(freT, 1.0)])

        # slot-0 scores for all b: w_alpha . tanh(fre + hoe)
        ha0 = misc.tile([P, KC * BC], BF16, tag="ha0")
        nc.vector.tensor_tensor(ha0[:], freT[:], hoeT[:], op=ALU.add)
        ta0 = misc.tile([P, KC * BC], FP8, tag="ta0")
        nc.scalar.activation(ta0[:], ha0[:], AFT.Tanh)
        s0ps = spsum.tile([1, A], F32, tag="sps")
        for c in range(KC):
            nc.tensor.matmul(
                s0ps[0:1, 0:BC],
                lhsT=wal[:, 16 * c : 16 * c + 1],
                rhs=ta0[:, c * BC : (c + 1) * BC],
                start=(c == 0),
                stop=(c == KC - 1),
            )
        # slot-0 exp weights (scores bounded, no max subtraction needed)
        e0all = singles.tile([1, BC], F32, tag="e0all")
        nc.scalar.activation(e0all[:], s0ps[0:1, 0:BC], AFT.Exp, scale=1.0 / WS)

        # ---------------- phase 2 (fused): scores -> row softmax -> vis ----------------
        # pim holds, per batch row b, a [128, 2, 32] fp8 block whose only
        # nonzero column b (in each slot k-tile) is 16*exp(score); accumulating
        # all b into shared [32, 512] psum tiles via DoubleRow matmuls yields
        # 256*unnormalized vis for every batch row.
        pim = singles.tile([P, BC * 2 * BC], FP8, tag="pim")
        nc.vector.memset(pim[:], 0.0)
        Zrow = singles.tile([1, BC], F32, tag="Zrow")

        vp = [mpsum.tile([BC, 512], F32, tag="mp", name=f"vp{ns}") for ns in range(NS)]

        def emit_vis(b, c01):
            cv = c01[:].rearrange("q (k n) -> q k n", n=D)
            pv = pim[:, b * 2 * BC : (b + 1) * 2 * BC].rearrange(
                "q (k m) -> q k m", m=BC
            )
            for ns in range(NS):
                nc.tensor.matmul(
                    vp[ns][:],
                    lhsT=pv,
                    rhs=cv[:, :, ns * 512 : (ns + 1) * 512],
                    start=(b == 0),
                    stop=(b == BC - 1),
                    perf_mode=DR,
                )

        # persistent conv_feat buffers with pre-zeroed pad rows (68:128 of slot
        # k-tile 1); per-iteration DMAs only write rows 0:128 (tile 0) and
        # 0:68 (tile 1), so the pad stays zero across reuse.
        NCF = 3
        c01_tiles = []
        for f in range(NCF):
            cz = singles.tile([P, 2 * D], FP8, tag=f"c01_{f}")
            nc.vector.memset(cz[64:P, D : 2 * D], 0.0)
            c01_tiles.append(cz)

        # Software-pipelined by one iteration: the vis matmuls for b-1 are
        # emitted between b's score matmuls and b's softmax-dependent
        # transposes, so the tensor engine never waits on the softmax tail.
        pending_vis = []
        wa2h_pre = []
        for b in range(BC):
            cfeb = cfepool.tile([P, KC * A], FP8, tag="cfeb")
            nc.sync.dma_start(cfeb[:], cfe_d[b])
            c01 = c01_tiles[b % NCF]
            nc.sync.dma_start(c01[:, 0:D], cf_d[b, 0:P, :])
            nc.sync.dma_start(c01[0:A1, D : 2 * D], cf_d[b, P:A, :])

            # th = tanh(cfe + hoe[b]) in fp8; the hoe bias rides the ACT
            # instruction (per-partition bias column), no DVE add needed
            th = thpool.tile([P, KC * A], FP8, tag="th")
            for k in range(KC):
                nc.scalar.activation(
                    th[:, k * A : (k + 1) * A],
                    cfeb[:, k * A : (k + 1) * A],
                    AFT.Tanh,
                    bias=hoeT[:, k * BC + b : k * BC + b + 1],
                    scale=1.0 / XS,
                )
            sps = spsum.tile([16, A], F32, tag="sps")
            for c in range(NP):
                wv = wal[:, 32 * c : 32 * (c + 1)].rearrange("q (k o) -> q k o", o=16)
                tv = th[:, 2 * c * A : (2 * c + 2) * A].rearrange(
                    "q (k a) -> q k a", a=A
                )
                nc.tensor.matmul(
                    sps[:],
                    lhsT=wv,
                    rhs=tv,
                    start=(c == 0),
                    stop=(c == NP - 1),
                    perf_mode=DR,
                )
            if pending_vis:
                emit_vis(*pending_vis.pop(0))
            # unnormalized exp weights straight from the score psum (scores
            # bounded, exp safe); zcell = sum over conv slots
            erow = rowpool.tile([1, A], BF16, tag="erow")
            zcell = rowpool.tile([1, 1], F32, tag="zcell")
            nc.scalar.activation(
                erow[:], sps[0:1, :], AFT.Exp, scale=1.0 / WS, accum_out=zcell[:]
            )
            nc.vector.tensor_copy(Zrow[0:1, b : b + 1], zcell[:])
            # transpose exp weights into the masked DoubleRow lhsT columns (x16)
            tp = tpsum.tile([P, 4], BF16, tag="tps", name=f"tp{b}")
            nc.tensor.transpose(tp[:, 0:1], erow[0:1, 0:P], id_bf[0:1, 0:1])
            nc.tensor.transpose(tp[0:A1, 2:3], erow[0:1, P:A], id_bf[0:1, 0:1])
            col0 = b * 2 * BC + b
            col1 = b * 2 * BC + BC + b
            nc.vector.tensor_scalar_mul(pim[:, col0 : col0 + 1], tp[:, 0:1], XS)
            nc.vector.tensor_scalar_mul(pim[0:A1, col1 : col1 + 1], tp[0:A1, 2:3], XS)
            pending_vis.append((b, c01))
            # prefetch the final-layer weight chunks late in the loop
            if b >= BC - 8:
                k = b - (BC - 8)
                wt = wpool.tile([P, D], BF16, tag="w", name=f"w_a2h{k}")
                nc.sync.dma_start(wt[:], wb_d["wa2h"][:, k * D : (k + 1) * D])
                wa2h_pre.append(wt)
        for pv_ in pending_vis:
            emit_vis(*pv_)

        # ---------------- phase 3: normalize, atten_out, final linear ----------------
        # total Z = conv-slot sum + slot-0 exp; 1/(256 Z) as a column for vis rows
        nc.vector.tensor_tensor(Zrow[:], Zrow[:], e0all[:], op=ALU.add)
        zt = tpsum.tile([P, 2], F32, tag="tps", name="zt")
        nc.tensor.transpose(zt[0:BC, 0:1], Zrow[:], id_f32[0:1, 0:1])
        zcol = singles.tile([BC, 1], F32, tag="zcol")
        nc.vector.tensor_scalar_mul(zcol[:], zt[0:BC, 0:1], XS * XS)
        rinv = singles.tile([BC, 1], F32, tag="rinv")
        nc.vector.reciprocal(rinv[:], zcol[:])           # 1/(256 Z)
        vis_bm = singles.tile([BC, D], F32, tag="vis_bm")
        for ns in range(NS):
            nc.vector.tensor_scalar_mul(
                vis_bm[:, ns * 512 : (ns + 1) * 512], vp[ns][:], rinv[:]
            )
        # pi0/16 = e0/(16 Z) broadcast to [128, BC] (the 1/16 cancels frT8's x16)
        zs = singles.tile([1, BC], F32, tag="zs")
        nc.vector.tensor_scalar_mul(zs[:], Zrow[:], XS)
        zrinv = singles.tile([1, BC], F32, tag="zrinv")
        nc.vector.reciprocal(zrinv[:], zs[:])
        e0z = singles.tile([1, BC], BF16, tag="e0z")
        nc.vector.tensor_tensor(e0z[:], e0all[:], zrinv[:], op=ALU.mult)
        pb = tpsum.tile([P, BC], F32, tag="tps", name="pb")
        nc.tensor.matmul(pb[:], lhsT=ones[0:1, 0:P], rhs=e0z[:], start=True, stop=True)
        pi0b = singles.tile([P, BC], F32, tag="pi0b")
        nc.vector.tensor_copy(pi0b[:], pb[:])

        # atten_out.T = vis.T + hol.T + (pi0/16)*(16 fr.T)  (feature-major, bf16)
        attT = singles.tile([P, KC * BC], BF16, tag="attT")
        for k in range(KC):
            vt = tpsum.tile([P, BC], F32, tag="tps", name=f"vt{k}")
            nc.tensor.transpose(vt[:], vis_bm[:, k * P : (k + 1) * P], id_f32[:])
            t1 = misc.tile([P, BC], F32, tag="t1")
            nc.vector.tensor_tensor(t1[:], vt[:], holT[:, k * BC : (k + 1) * BC], op=ALU.add)
            t2 = misc.tile([P, BC], F32, tag="t2")
            nc.vector.tensor_tensor(t2[:], pi0b[:], frT8[:, k * BC : (k + 1) * BC], op=ALU.mult)
            nc.vector.tensor_tensor(
                attT[:, k * BC : (k + 1) * BC], t1[:], t2[:], op=ALU.add
            )

        # final linear: out = tanh(atten @ W_a2h.T + b)
        ps = [mpsum.tile([BC, 512], F32, tag="mp", name=f"fps{ns}") for ns in range(NS)]
        for k in range(KC):
            if k < len(wa2h_pre):
                wt = wa2h_pre[k]
            else:
                wt = wpool.tile([P, D], BF16, tag="w", name=f"w_a2h{k}")
                nc.sync.dma_start(wt[:], wb_d["wa2h"][:, k * D : (k + 1) * D])
            for ns in range(NS):
                nc.tensor.matmul(
                    ps[ns][:],
                    lhsT=attT[:, k * BC : (k + 1) * BC],
                    rhs=wt[:, ns * 512 : (ns + 1) * 512],
                    start=(k == 0),
                    stop=False,
                )
        outsb = singles.tile([BC, D], F32, tag="outsb")
        for ns in range(NS):
            nc.tensor.matmul(
                ps[ns][:],
                lhsT=ones[0:1, 0:BC],
                rhs=b_sb["ba2h"][0:1, ns * 512 : (ns + 1) * 512],
                start=False,
                stop=True,
            )
            nc.scalar.activation(outsb[:, ns * 512 : (ns + 1) * 512], ps[ns][:], AFT.Tanh)
        nc.sync.dma_start(out_d[:], outsb[:])

    nc.compile()
    return nc


def _bf16(x):
    return np.ascontiguousarray(np.asarray(x, dtype=np.float32).astype(ml_dtypes.bfloat16))


def _fp8(x):
    return np.ascontiguousarray(np.asarray(x, dtype=np.float32).astype(ml_dtypes.float8_e4m3))


def _chunked_wT(W):
    # W [D, D] (out_features, in_features) -> W.T chunk layout [128, KC*D] f32
    Wt = np.asarray(W, dtype=np.float32).T  # [k, n]
    return Wt.reshape(KC, P, D).transpose(1, 0, 2).reshape(P, KC * D)


def _prep_xT(x, scale):
    # x [BC, D] -> X.T chunk layout [128, KC*BC] f32
    t = np.asarray(x, dtype=np.float32).T * scale  # [D, BC]
    return t.reshape(KC, P, BC).transpose(1, 0, 2).reshape(P, KC * BC)


def _prep_cfe(e):
    # e [BC, A, D] -> [BC, 128, KC*A] with chunk c at free offset c*A, fp8 (x16)
    t = np.asarray(e, dtype=np.float32).transpose(0, 2, 1) * XS  # [BC, D, A]
    r = t.reshape(BC, KC, P, A).transpose(0, 2, 1, 3).reshape(BC, P, KC * A)
    return _fp8(r)


def _prep_walpha(w):
    # [D] -> [128, KC*16] fp8 (x64): chunk k lives in col 16k, rest zero
    wc = np.asarray(w, dtype=np.float32).reshape(KC, P).T * WS  # [128, KC]
    out = np.zeros((P, KC * 16), dtype=np.float32)
    out[:, ::16] = wc
    return _fp8(out)


def _make_in_maps(inputs):
    h_out = np.asarray(inputs["h_out"], dtype=np.float32)
    fake_region = np.asarray(inputs["fake_region"], dtype=np.float32)
    conv_feat = np.asarray(inputs["conv_feat"], dtype=np.float32)
    conv_feat_embed = np.asarray(inputs["conv_feat_embed"], dtype=np.float32)

    shared = {
        "wfr": _fp8(_chunked_wT(inputs["W_fr"]) * WS),
        "wfre": _fp8(_chunked_wT(inputs["W_fre"]) * WS),
        "whoe": _fp8(_chunked_wT(inputs["W_hoe"]) * WS),
        "who": _bf16(_chunked_wT(inputs["W_ho"])),
        "wa2h": _bf16(_chunked_wT(inputs["W_a2h"])),
        # biases pre-scaled to psum scale (see linear_* helpers)
        "bfr": _bf16(np.asarray(inputs["b_fr"]).reshape(1, D) * (XS * WS)),
        "bfre": _bf16(np.asarray(inputs["b_fre"]).reshape(1, D) * (XS * WS)),
        "bho": _bf16(np.asarray(inputs["b_ho"]).reshape(1, D)),
        "bhoe": _bf16(np.asarray(inputs["b_hoe"]).reshape(1, D) * (XS * WS)),
        "ba2h": _bf16(np.asarray(inputs["b_a2h"]).reshape(1, D)),
        "walpha": _prep_walpha(inputs["w_alpha"]),
    }
    in_maps = []
    for c in range(NCORES):
        sl = slice(c * BC, (c + 1) * BC)
        in_maps.append(
            dict(
                shared,
                xfr=_fp8(_prep_xT(fake_region[sl], XS)),
                xho=_bf16(_prep_xT(h_out[sl], 1.0)),
                cfe=_prep_cfe(conv_feat_embed[sl]),
                cf=_fp8(conv_feat[sl] * XS),
            )
        )
    return in_maps


def _run(inputs, trace=False):
    if "nc" not in _CACHE:
        _CACHE["nc"] = _build_graph()
    nc = _CACHE["nc"]
    in_maps = _make_in_maps(inputs)
    res = run_bass_kernel_spmd(nc, in_maps, core_ids=list(range(NCORES)), trace=trace)
    out = np.concatenate([r["out"] for r in res.results], axis=0)
    return out.astype(np.float32), res


def kernel(**inputs):
    out, _ = _run(inputs, trace=False)
    return out


# revision 26
# speedup vs baseline: 1.4411x; 1.2792x over previous
"""AdaAtt attention kernel for Trainium2 (8 NeuronCores, data-parallel over batch).

v2: fp8-heavy. Per-core HBM bytes cut from ~81MB to ~56MB:
  - conv_feat fp8 (x16 scale), conv_feat_embed fp8 (x16)
  - W_fr / W_fre / W_hoe fp8 (x64) driven in DoubleRow perf mode
  - W_ho / W_a2h stay bf16 (fp8 there fails the 2e-2 tolerance)
  - tanh'd score operand + exp attention weights fp8 -> DoubleRow scores/vis
  - bias-add of hoe into conv_feat_embed fused into the ACT tanh (bias arg),
    eliminating the 12.8M-element DVE broadcast add

Self-contained: takes full inputs (as produced by the problem's setup_inputs),
shards batch across 8 cores, runs a Bass/Tile kernel via run_bass_kernel_spmd,
and returns the full [256, 2048] float32 output.
"""

from contextlib import ExitStack

import ml_dtypes
import numpy as np

import concourse.bass as bass
import concourse.mybir as mybir
import concourse.tile as tile
from concourse import bacc
from concourse.bass_utils import run_bass_kernel_spmd
from concourse.masks import make_identity

# Problem dims (hardcoded per spec)
B, A, D = 256, 196, 2048
NCORES = 8
BC = B // NCORES          # 32 batch rows per core
P = 128
KC = D // P               # 16 feature chunks
NP = KC // 2              # 8 DoubleRow chunk pairs
NS = D // 512             # 4 psum n-slices
A1 = A - P                # 68 rows in second conv chunk

XS = 16.0                 # fp8 scale for inputs/activations
WS = 64.0                 # fp8 scale for weights

F32 = mybir.dt.float32
BF16 = mybir.dt.bfloat16
FP8 = mybir.dt.float8e4
AFT = mybir.ActivationFunctionType
ALU = mybir.AluOpType
DR = mybir.MatmulPerfMode.DoubleRow

_CACHE = {}


def _build_graph():
    nc = bacc.Bacc("TRN2")

    # ---------------- DRAM parameters ----------------
    xfr_d = nc.dram_tensor("xfr", [P, KC * BC], FP8, kind="ExternalInput")   # 16*fake_region.T
    xho_d = nc.dram_tensor("xho", [P, KC * BC], BF16, kind="ExternalInput")  # h_out.T
    cfe_d = nc.dram_tensor("cfe", [BC, P, KC * A], FP8, kind="ExternalInput")  # 16*cfe
    # conv_feat packed for DoubleRow: [:, 0:D] = slots 0:128, [:, D:2D] rows
    # 0:68 = slots 128:196 (rest zero-padded on host). One uniform 128-row
    # DMA per batch row keeps the 16 DMA engines evenly loaded.
    cf_d = nc.dram_tensor("cf", [BC, P, 2 * D], FP8, kind="ExternalInput")     # 16*cf
    w8_d = {
        name: nc.dram_tensor(name, [P, KC * D], FP8, kind="ExternalInput")     # 64*W.T
        for name in ["wfr", "wfre", "whoe"]
    }
    wb_d = {
        name: nc.dram_tensor(name, [P, KC * D], BF16, kind="ExternalInput")    # W.T
        for name in ["who", "wa2h"]
    }
    # biases pre-scaled on host to match their psum scales
    b_d = {
        name: nc.dram_tensor(name, [1, D], BF16, kind="ExternalInput")
        for name in ["bfr", "bfre", "bho", "bhoe", "ba2h"]
    }
    # w_alpha padded to 16 cols per chunk (col 0 real, rest zero): DoubleRow
    # ldweights requires k-tile step % 16 == 0
    wal_d = nc.dram_tensor("walpha", [P, KC * 16], FP8, kind="ExternalInput")  # 64*w_alpha
    out_d = nc.dram_tensor("out", [BC, D], F32, kind="ExternalOutput")

    with ExitStack() as ctx:
        tc = ctx.enter_context(tile.TileContext(nc))

        singles = ctx.enter_context(tc.tile_pool(name="singles", bufs=1))
        wpool = ctx.enter_context(tc.tile_pool(name="wpool", bufs=8))
        bmpool = ctx.enter_context(tc.tile_pool(name="bm", bufs=2))
        cfepool = ctx.enter_context(tc.tile_pool(name="cfep", bufs=3))
        thpool = ctx.enter_context(tc.tile_pool(name="thp", bufs=3))
        habpool = ctx.enter_context(tc.tile_pool(name="habp", bufs=3))
        cfpool = ctx.enter_context(tc.tile_pool(name="cfp", bufs=3))
        misc = ctx.enter_context(tc.tile_pool(name="misc", bufs=2))
        rowpool = ctx.enter_context(tc.tile_pool(name="rows", bufs=4))

        mpsum = ctx.enter_context(tc.tile_pool(name="mpsum", bufs=4, space="PSUM"))
        tpsum = ctx.enter_context(tc.tile_pool(name="tpsum", bufs=2, space="PSUM"))
        spsum = ctx.enter_context(tc.tile_pool(name="spsum", bufs=2, space="PSUM"))

        # ---------------- constants / small inputs ----------------
        ones = singles.tile([1, P], BF16, tag="ones")
        nc.vector.memset(ones[:], 1.0)
        id_bf = singles.tile([BC, BC], BF16, tag="id_bf")
        make_identity(nc, id_bf[:])
        id_f32 = singles.tile([BC, BC], F32, tag="id_f32")
        make_identity(nc, id_f32[:])

        xfr = singles.tile([P, KC * BC], FP8, tag="xfr")
        nc.sync.dma_start(xfr[:], xfr_d[:])
        xho = singles.tile([P, KC * BC], BF16, tag="xho")
        nc.sync.dma_start(xho[:], xho_d[:])
        wal = singles.tile([P, KC * 16], FP8, tag="wal")
        nc.sync.dma_start(wal[:], wal_d[:])
        b_sb = {}
        for name in b_d:
            t = singles.tile([1, D], BF16, tag=name)
            nc.sync.dma_start(t[:], b_d[name][:])
            b_sb[name] = t

        # ---------------- helpers ----------------
        def linear_bf16(x_lhsT, wname, bname, act, out_bm, scale=1.0):
            """out_bm[BC, D] = act((x @ W.T + b)*scale); bf16 weights streamed
            as k-chunk pairs, all 4 psum n-slices accumulating concurrently."""
            ps = [mpsum.tile([BC, 512], F32, tag="mp", name=f"mp_{wname}{ns}") for ns in range(NS)]
            for p in range(NP):
                wt = wpool.tile([P, 2 * D], BF16, tag="w", name=f"w_{wname}{p}")
                nc.sync.dma_start(wt[:], wb_d[wname][:, 2 * p * D : (2 * p + 2) * D])
                for kk in range(2):
                    k = 2 * p + kk
                    for ns in range(NS):
                        nc.tensor.matmul(
                            ps[ns][:],
                            lhsT=x_lhsT[:, k * BC : (k + 1) * BC],
                            rhs=wt[:, kk * D + ns * 512 : kk * D + (ns + 1) * 512],
                            start=(k == 0),
                            stop=False,
                        )
            for ns in range(NS):
                nc.tensor.matmul(
                    ps[ns][:],
                    lhsT=ones[0:1, 0:BC],
                    rhs=b_sb[bname][0:1, ns * 512 : (ns + 1) * 512],
                    start=False,
                    stop=True,
                )
                nc.scalar.activation(out_bm[:, ns * 512 : (ns + 1) * 512], ps[ns][:], act, scale=scale)
            return out_bm

        def linear_dr(x8_lhsT, wname, bname, act, out_bm, scale):
            """out_bm[BC, D] = act((psum + b)*scale) with fp8 DoubleRow matmuls.
            x8_lhsT [P, KC*BC] fp8, weight [P, KC*D] fp8; psum = xscale*wscale*(x@W.T)."""
            ps = [mpsum.tile([BC, 512], F32, tag="mp", name=f"mp_{wname}{ns}") for ns in range(NS)]
            for q in range(NP // 2):
                wt = wpool.tile([P, 4 * D], FP8, tag="w", name=f"w_{wname}{q}")
                nc.sync.dma_start(wt[:], w8_d[wname][:, 4 * q * D : (4 * q + 4) * D])
                for pp in range(2):
                    p = 2 * q + pp
                    wv = wt[:, pp * 2 * D : (pp + 1) * 2 * D].rearrange(
                        "q (k n) -> q k n", n=D
                    )
                    xv = x8_lhsT[:, 2 * p * BC : (2 * p + 2) * BC].rearrange(
                        "q (k m) -> q k m", m=BC
                    )
                    for ns in range(NS):
                        nc.tensor.matmul(
                            ps[ns][:],
                            lhsT=xv,
                            rhs=wv[:, :, ns * 512 : (ns + 1) * 512],
                            start=(p == 0),
                            stop=False,
                            perf_mode=DR,
                        )
            for ns in range(NS):
                nc.tensor.matmul(
                    ps[ns][:],
                    lhsT=ones[0:1, 0:BC],
                    rhs=b_sb[bname][0:1, ns * 512 : (ns + 1) * 512],
                    start=False,
                    stop=True,
                )
                nc.scalar.activation(out_bm[:, ns * 512 : (ns + 1) * 512], ps[ns][:], act, scale=scale)
            return out_bm

        def to_feature_major(bm, outs, in_f32=False):
            """bm [BC, D] -> feature-major [P, KC*BC] via PE transposes.
            outs: list of (tile, mul) to produce (mul applied on DVE, dtype
            conversion via the copy)."""
            ident = id_f32 if in_f32 else id_bf
            dt = F32 if in_f32 else BF16
            for k in range(KC):
                pt = tpsum.tile([P, BC], dt, tag="tps", name=f"pt_{outs[0][0].name}{k}")
                nc.tensor.transpose(pt[:], bm[:, k * P : (k + 1) * P], ident[:])
                for t, mul in outs:
                    if mul == 1.0:
                        nc.vector.tensor_copy(t[:, k * BC : (k + 1) * BC], pt[:])
                    else:
                        nc.vector.tensor_scalar_mul(
                            t[:, k * BC : (k + 1) * BC], pt[:], mul
                        )

        # ---------------- phase 1: front linears ----------------
        # ho-chain first: hoeT is the only input the fused attention loop needs.
        hol_bm = bmpool.tile([BC, D], BF16, tag="bm", name="hol_bm")
        linear_bf16(xho, "who", "bho", AFT.Tanh, hol_bm)
        holT = singles.tile([P, KC * BC], BF16, tag="holT")
        hol8T = singles.tile([P, KC * BC], FP8, tag="hol8T")   # 16*hol
        to_feature_major(hol_bm, [(holT, 1.0), (hol8T, XS)])

        hoe_bm = bmpool.tile([BC, D], BF16, tag="bm", name="hoe_bm")
        linear_dr(hol8T, "whoe", "bhoe", AFT.Copy, hoe_bm, scale=1.0 / (XS * WS))
        hoeT = singles.tile([P, KC * BC], BF16, tag="hoeT")
        hoe16T = singles.tile([P, KC * BC], BF16, tag="hoe16T")      # 16*hoe
        to_feature_major(hoe_bm, [(hoeT, 1.0), (hoe16T, XS)])

        frT8 = singles.tile([P, KC * BC], FP8, tag="frT8")           # 16*fr
        freT = singles.tile([P, KC * BC], BF16, tag="freT")
        e0all = singles.tile([1, BC], F32, tag="e0all")

        def fr_chain():
            """fr/fre linears + slot-0 score; emitted mid-loop so their weight
            streams overlap the attention loop instead of delaying it."""
            fr_bm = bmpool.tile([BC, D], BF16, tag="bm", name="fr_bm")   # 16*fr
            linear_dr(xfr, "wfr", "bfr", AFT.Relu, fr_bm, scale=1.0 / WS)
            to_feature_major(fr_bm, [(frT8, 1.0)])

            fre_bm = bmpool.tile([BC, D], BF16, tag="bm", name="fre_bm")
            linear_dr(frT8, "wfre", "bfre", AFT.Copy, fre_bm, scale=1.0 / (XS * WS))
            to_feature_major(fre_bm, [(freT, 1.0)])

            # slot-0 scores for all b: w_alpha . tanh(fre + hoe)
            ha0 = misc.tile([P, KC * BC], BF16, tag="ha0")
            nc.vector.tensor_tensor(ha0[:], freT[:], hoeT[:], op=ALU.add)
            ta0 = misc.tile([P, KC * BC], FP8, tag="ta0")
            nc.scalar.activation(ta0[:], ha0[:], AFT.Tanh)
            s0ps = spsum.tile([1, A], F32, tag="sps", name="s0ps")
            for c in range(KC):
                nc.tensor.matmul(
                    s0ps[0:1, 0:BC],
                    lhsT=wal[:, 16 * c : 16 * c + 1],
                    rhs=ta0[:, c * BC : (c + 1) * BC],
                    start=(c == 0),
                    stop=(c == KC - 1),
                )
            # slot-0 exp weights (scores bounded, no max subtraction needed)
            nc.scalar.activation(e0all[:], s0ps[0:1, 0:BC], AFT.Exp, scale=1.0 / WS)

        # ---------------- phase 2 (fused): scores -> row softmax -> vis ----------------
        # pim holds, per batch row b, a [128, 2, 32] fp8 block whose only
        # nonzero column b (in each slot k-tile) is 16*exp(score); accumulating
        # all b into shared [32, 512] psum tiles via DoubleRow matmuls yields
        # 256*unnormalized vis for every batch row.
        pim = singles.tile([P, BC * 2 * BC], FP8, tag="pim")
        nc.vector.memset(pim[:], 0.0)
        Zrow = singles.tile([1, BC], F32, tag="Zrow")

        # vp allocated lazily at the first emit_vis (b==2) so fr_chain's
        # psum tiles (emitted at b==1) don't collide with it in the pool
        vp = []

        def emit_vis(b, c01):
            if not vp:
                vp.extend(
                    mpsum.tile([BC, 512], F32, tag="mp", name=f"vp{ns}")
                    for ns in range(NS)
                )
            cv = c01[:].rearrange("q (k n) -> q k n", n=D)
            pv = pim[:, b * 2 * BC : (b + 1) * 2 * BC].rearrange(
                "q (k m) -> q k m", m=BC
            )
            for ns in range(NS):
                nc.tensor.matmul(
                    vp[ns][:],
                    lhsT=pv,
                    rhs=cv[:, :, ns * 512 : (ns + 1) * 512],
                    start=(b == 0),
                    stop=(b == BC - 1),
                    perf_mode=DR,
                )

        # Software-pipelined by one iteration: the vis matmuls for b-1 are
        # emitted between b's score matmuls and b's softmax-dependent
        # transposes, so the tensor engine never waits on the softmax tail.
        pending_vis = []
        wa2h_pre = []
        NH = 2
        HK = KC // NH
        for b in range(BC):
            cfeb = cfepool.tile([P, KC * A], FP8, tag="cfeb")
            nc.sync.dma_start(cfeb[:], cfe_d[b])
            c01 = cfpool.tile([P, 2 * D], FP8, tag="c01")
            nc.sync.dma_start(c01[:], cf_d[b])

            # th = tanh(cfe + hoe[b]): broadcast add on DVE (in x16 fp8 space),
            # tanh on ACT in two halves so they overlap
            hab = habpool.tile([P, KC * A], FP8, tag="hab")
            th = thpool.tile([P, KC * A], FP8, tag="th")
            for h in range(NH):
                hview = (
                    hoe16T[:]
                    .rearrange("p (c q) -> p c q", q=BC)[:, h * HK : (h + 1) * HK, b : b + 1]
                    .to_broadcast((P, HK, A))
                )
                sl = slice(h * HK * A, (h + 1) * HK * A)
                nc.vector.tensor_tensor(
                    hab[:, sl].rearrange("p (c a) -> p c a", a=A),
                    cfeb[:, sl].rearrange("p (c a) -> p c a", a=A),
                    hview,
                    op=ALU.add,
                )
                nc.scalar.activation(th[:, sl], hab[:, sl], AFT.Tanh, scale=1.0 / XS)
            sps = spsum.tile([16, A], F32, tag="sps")
            for c in range(NP):
                wv = wal[:, 32 * c : 32 * (c + 1)].rearrange("q (k o) -> q k o", o=16)
                tv = th[:, 2 * c * A : (2 * c + 2) * A].rearrange(
                    "q (k a) -> q k a", a=A
                )
                nc.tensor.matmul(
                    sps[:],
                    lhsT=wv,
                    rhs=tv,
                    start=(c == 0),
                    stop=(c == NP - 1),
                    perf_mode=DR,
                )
            if len(pending_vis) >= 2:
                emit_vis(*pending_vis.pop(0))
            # unnormalized exp weights straight from the score psum (scores
            # bounded, exp safe); zcell = sum over conv slots
            erow = rowpool.tile([1, A], BF16, tag="erow")
            zcell = rowpool.tile([1, 1], F32, tag="zcell")
            nc.scalar.activation(
                erow[:], sps[0:1, :], AFT.Exp, scale=1.0 / WS, accum_out=zcell[:]
            )
            nc.vector.tensor_copy(Zrow[0:1, b : b + 1], zcell[:])
            # transpose exp weights into the masked DoubleRow lhsT columns (x16)
            tp = tpsum.tile([P, 4], BF16, tag="tps", name=f"tp{b}")
            nc.tensor.transpose(tp[:, 0:1], erow[0:1, 0:P], id_bf[0:1, 0:1])
            nc.tensor.transpose(tp[0:A1, 2:3], erow[0:1, P:A], id_bf[0:1, 0:1])
            col0 = b * 2 * BC + b
            col1 = b * 2 * BC + BC + b
            nc.vector.tensor_scalar_mul(pim[:, col0 : col0 + 1], tp[:, 0:1], XS)
            nc.vector.tensor_scalar_mul(pim[0:A1, col1 : col1 + 1], tp[0:A1, 2:3], XS)
            pending_vis.append((b, c01))
            if b == 1:
                fr_chain()
            # prefetch the final-layer weight chunks late in the loop
            if b >= BC - 8:
                k = b - (BC - 8)
                wt = wpool.tile([P, D], BF16, tag="w", name=f"w_a2h{k}")
                nc.sync.dma_start(wt[:], wb_d["wa2h"][:, k * D : (k + 1) * D])
                wa2h_pre.append(wt)
        for pv_ in pending_vis:
            emit_vis(*pv_)

        # ---------------- phase 3: normalize, atten_out, final linear ----------------
        # total Z = conv-slot sum + slot-0 exp; 1/(256 Z) as a column for vis rows
        nc.vector.tensor_tensor(Zrow[:], Zrow[:], e0all[:], op=ALU.add)
        zt = tpsum.tile([P, 2], F32, tag="tps", name="zt")
        nc.tensor.transpose(zt[0:BC, 0:1], Zrow[:], id_f32[0:1, 0:1])
        zcol = singles.tile([BC, 1], F32, tag="zcol")
        nc.vector.tensor_scalar_mul(zcol[:], zt[0:BC, 0:1], XS * XS)
        rinv = singles.tile([BC, 1], F32, tag="rinv")
        nc.vector.reciprocal(rinv[:], zcol[:])           # 1/(256 Z)
        vis_bm = singles.tile([BC, D], F32, tag="vis_bm")
        for ns in range(NS):
            nc.vector.tensor_scalar_mul(
                vis_bm[:, ns * 512 : (ns + 1) * 512], vp[ns][:], rinv[:]
            )
        # pi0/16 = e0/(16 Z) broadcast to [128, BC] (the 1/16 cancels frT8's x16)
        zs = singles.tile([1, BC], F32, tag="zs")
        nc.vector.tensor_scalar_mul(zs[:], Zrow[:], XS)
        zrinv = singles.tile([1, BC], F32, tag="zrinv")
        nc.vector.reciprocal(zrinv[:], zs[:])
        e0z = singles.tile([1, BC], BF16, tag="e0z")
        nc.vector.tensor_tensor(e0z[:], e0all[:], zrinv[:], op=ALU.mult)
        pb = tpsum.tile([P, BC], F32, tag="tps", name="pb")
        nc.tensor.matmul(pb[:], lhsT=ones[0:1, 0:P], rhs=e0z[:], start=True, stop=True)
        pi0b = singles.tile([P, BC], F32, tag="pi0b")
        nc.vector.tensor_copy(pi0b[:], pb[:])

        # atten_out.T = vis.T + hol.T + (pi0/16)*(16 fr.T)  (feature-major, bf16)
        attT = singles.tile([P, KC * BC], BF16, tag="attT")
        for k in range(KC):
            vt = tpsum.tile([P, BC], F32, tag="tps", name=f"vt{k}")
            nc.tensor.transpose(vt[:], vis_bm[:, k * P : (k + 1) * P], id_f32[:])
            t1 = misc.tile([P, BC], F32, tag="t1")
            nc.vector.tensor_tensor(t1[:], vt[:], holT[:, k * BC : (k + 1) * BC], op=ALU.add)
            t2 = misc.tile([P, BC], F32, tag="t2")
            nc.vector.tensor_tensor(t2[:], pi0b[:], frT8[:, k * BC : (k + 1) * BC], op=ALU.mult)
            nc.vector.tensor_tensor(
                attT[:, k * BC : (k + 1) * BC], t1[:], t2[:], op=ALU.add
            )

        # final linear: out = tanh(atten @ W_a2h.T + b)
        ps = [mpsum.tile([BC, 512], F32, tag="mp", name=f"fps{ns}") for ns in range(NS)]
        for k in range(KC):
            if k < len(wa2h_pre):
                wt = wa2h_pre[k]
            else:
                wt = wpool.tile([P, D], BF16, tag="w", name=f"w_a2h{k}")
                nc.sync.dma_start(wt[:], wb_d["wa2h"][:, k * D : (k + 1) * D])
            for ns in range(NS):
                nc.tensor.matmul(
                    ps[ns][:],
                    lhsT=attT[:, k * BC : (k + 1) * BC],
                    rhs=wt[:, ns * 512 : (ns + 1) * 512],
                    start=(k == 0),
                    stop=False,
                )
        outsb = singles.tile([BC, D], F32, tag="outsb")
        for ns in range(NS):
            nc.tensor.matmul(
                ps[ns][:],
                lhsT=ones[0:1, 0:BC],
                rhs=b_sb["ba2h"][0:1, ns * 512 : (ns + 1) * 512],
                start=False,
                stop=True,
            )
            nc.scalar.activation(outsb[:, ns * 512 : (ns + 1) * 512], ps[ns][:], AFT.Tanh)
        nc.sync.dma_start(out_d[:], outsb[:])

    nc.compile()
    return nc


def _bf16(x):
    return np.ascontiguousarray(np.asarray(x, dtype=np.float32).astype(ml_dtypes.bfloat16))


def _fp8(x):
    return np.ascontiguousarray(np.asarray(x, dtype=np.float32).astype(ml_dtypes.float8_e4m3))


def _chunked_wT(W):
    # W [D, D] (out_features, in_features) -> W.T chunk layout [128, KC*D] f32
    Wt = np.asarray(W, dtype=np.float32).T  # [k, n]
    return Wt.reshape(KC, P, D).transpose(1, 0, 2).reshape(P, KC * D)


def _prep_xT(x, scale):
    # x [BC, D] -> X.T chunk layout [128, KC*BC] f32
    t = np.asarray(x, dtype=np.float32).T * scale  # [D, BC]
    return t.reshape(KC, P, BC).transpose(1, 0, 2).reshape(P, KC * BC)


def _prep_cfe(e):
    # e [BC, A, D] -> [BC, 128, KC*A] with chunk c at free offset c*A, fp8 (x16)
    t = np.asarray(e, dtype=np.float32).transpose(0, 2, 1) * XS  # [BC, D, A]
    r = t.reshape(BC, KC, P, A).transpose(0, 2, 1, 3).reshape(BC, P, KC * A)
    return _fp8(r)


def _prep_cf(cf):
    # cf [BC, A, D] -> [BC, 128, 2*D] fp8 (x16): [:, :, 0:D] = slots 0:128,
    # [:, 0:68, D:2D] = slots 128:196, pad rows zero
    t = np.asarray(cf, dtype=np.float32) * XS
    out = np.zeros((BC, P, 2 * D), dtype=np.float32)
    out[:, :, 0:D] = t[:, 0:P, :]
    out[:, 0:A1, D : 2 * D] = t[:, P:A, :]
    return _fp8(out)


def _prep_walpha(w):
    # [D] -> [128, KC*16] fp8 (x64): chunk k lives in col 16k, rest zero
    wc = np.asarray(w, dtype=np.float32).reshape(KC, P).T * WS  # [128, KC]
    out = np.zeros((P, KC * 16), dtype=np.float32)
    out[:, ::16] = wc
    return _fp8(out)


def _make_in_maps(inputs):
    h_out = np.asarray(inputs["h_out"], dtype=np.float32)
    fake_region = np.asarray(inputs["fake_region"], dtype=np.float32)
    conv_feat = np.asarray(inputs["conv_feat"], dtype=np.float32)
    conv_feat_embed = np.asarray(inputs["conv_feat_embed"], dtype=np.float32)

    shared = {
        "wfr": _fp8(_chunked_wT(inputs["W_fr"]) * WS),
        "wfre": _fp8(_chunked_wT(inputs["W_fre"]) * WS),
        "whoe": _fp8(_chunked_wT(inputs["W_hoe"]) * WS),
        "who": _bf16(_chunked_wT(inputs["W_ho"])),
        "wa2h": _bf16(_chunked_wT(inputs["W_a2h"])),
        # biases pre-scaled to psum scale (see linear_* helpers)
        "bfr": _bf16(np.asarray(inputs["b_fr"]).reshape(1, D) * (XS * WS)),
        "bfre": _bf16(np.asarray(inputs["b_fre"]).reshape(1, D) * (XS * WS)),
        "bho": _bf16(np.asarray(inputs["b_ho"]).reshape(1, D)),
        "bhoe": _bf16(np.asarray(inputs["b_hoe"]).reshape(1, D) * (XS * WS)),
        "ba2h": _bf16(np.asarray(inputs["b_a2h"]).reshape(1, D)),
        "walpha": _prep_walpha(inputs["w_alpha"]),
    }
    in_maps = []
    for c in range(NCORES):
        sl = slice(c * BC, (c + 1) * BC)
        in_maps.append(
            dict(
                shared,
                xfr=_fp8(_prep_xT(fake_region[sl], XS)),
                xho=_bf16(_prep_xT(h_out[sl], 1.0)),
                cfe=_prep_cfe(conv_feat_embed[sl]),
                cf=_prep_cf(conv_feat[sl]),
            )
        )
    return in_maps


def _run(inputs, trace=False):
    if "nc" not in _CACHE:
        _CACHE["nc"] = _build_graph()
    nc = _CACHE["nc"]
    in_maps = _make_in_maps(inputs)
    res = run_bass_kernel_spmd(nc, in_maps, core_ids=list(range(NCORES)), trace=trace)
    out = np.concatenate([r["out"] for r in res.results], axis=0)
    return out.astype(np.float32), res


def kernel(**inputs):
    out, _ = _run(inputs, trace=False)
    return out
